# revision 3
# baseline (speedup 1.0000x reference)
"""Multi-head attention (B=4, S=2048, H=1024, NH=16) on 8 trn2 NeuronCores.

Sharding: tensor-parallel over heads. Core c owns heads 2c, 2c+1 (feature
columns 128c:128c+128 of Q/K/V). Each core computes its head-slice
projections from the full (host-pre-transposed) X^T, attention for its 8
(batch, head) pairs, and a rank-128 partial of the output projection.
Host sums the 8 partial O^T arrays, transposes back, and adds bo.

Device-side layout notes:
 - All activations are kept "transposed" (feature on partitions): Q^T/K^T
   are [128, 2048] per batch; scores are computed as S^T = K Q^T with
   k-positions on partitions so the additive mask is a per-partition bias
   fused into the ACT exp (softmax denominators come from an extra ones
   column appended to V: the PV matmul's 65th output row).
 - The two heads' QK matmuls (contraction = head_dim = 64) are row-tiled
   into PE partition halves 0:64 / 64:128 so they run concurrently.
 - Normalization: reciprocal of denom row -> broadcast over 64 partitions
   via a K=1 matmul with a ones stationary -> DVE multiply.
"""

import numpy as np

H = 1024
NH = 16
HD = 64
B = 4
S = 2048
N_CORES = 8
F = H // N_CORES  # 128 features (2 heads) per core
SBLK = 512  # s-block (moving-operand free dim, fp32 max)
NSB = S // SBLK  # 4 s-blocks per batch
NKT = S // 128  # 16 k-position tiles per batch

_nc_cache = {}


def _build_nc(apply_bv: bool):
    import concourse.bacc as bacc
    import concourse.tile as tile
    from concourse import mybir

    fp32 = mybir.dt.float32
    Act = mybir.ActivationFunctionType
    Alu = mybir.AluOpType
    ts = __import__("concourse.bass", fromlist=["ts"]).ts
    ds = __import__("concourse.bass", fromlist=["ds"]).ds

    nc = bacc.Bacc("TRN2", target_bir_lowering=False)

    xt = nc.dram_tensor("xt", [H, B * S], fp32, kind="ExternalInput")
    wqT = nc.dram_tensor("wqT", [H, F], fp32, kind="ExternalInput")
    wkT = nc.dram_tensor("wkT", [H, F], fp32, kind="ExternalInput")
    wvT = nc.dram_tensor("wvT", [H, F], fp32, kind="ExternalInput")
    woT = nc.dram_tensor("woT", [F, H], fp32, kind="ExternalInput")
    bqc = nc.dram_tensor("bqc", [F, 1], fp32, kind="ExternalInput")
    bkc = nc.dram_tensor("bkc", [F, 1], fp32, kind="ExternalInput")
    bvc = nc.dram_tensor("bvc", [F, 1], fp32, kind="ExternalInput")
    maskb = nc.dram_tensor("maskb", [B, S], fp32, kind="ExternalInput")
    outT = nc.dram_tensor("outT", [H, B * S], fp32, kind="ExternalOutput")

    xt_r = xt.rearrange("(t p) n -> p t n", p=128)  # [128, 8, 8192]

    from contextlib import ExitStack

    with tile.TileContext(nc) as tc, ExitStack() as es:
        const = es.enter_context(tc.tile_pool(name="const", bufs=1))
        maskp = es.enter_context(tc.tile_pool(name="maskp", bufs=2))
        xtp = es.enter_context(tc.tile_pool(name="xtp", bufs=2))
        qtp = es.enter_context(tc.tile_pool(name="qtp", bufs=2))
        ktp = es.enter_context(tc.tile_pool(name="ktp", bufs=2))
        vp = es.enter_context(tc.tile_pool(name="vp", bufs=2))
        pp = es.enter_context(tc.tile_pool(name="pp", bufs=18))
        ctxp = es.enter_context(tc.tile_pool(name="ctxp", bufs=2))
        cup = es.enter_context(tc.tile_pool(name="cup", bufs=4))
        rp = es.enter_context(tc.tile_pool(name="rp", bufs=4))
        osb = es.enter_context(tc.tile_pool(name="osb", bufs=4))
        ps_s = es.enter_context(tc.tile_pool(name="ps_s", bufs=3, space="PSUM"))
        ps_ctx = es.enter_context(tc.tile_pool(name="ps_ctx", bufs=2, space="PSUM"))
        ps_b = es.enter_context(tc.tile_pool(name="ps_b", bufs=1, space="PSUM"))
        ps_v = es.enter_context(tc.tile_pool(name="ps_v", bufs=1, space="PSUM"))
        ps_o = es.enter_context(tc.tile_pool(name="ps_o", bufs=1, space="PSUM"))

        if True:
            # --- constants ---
            wq_sb = const.tile([128, 8, F], fp32)
            wk_sb = const.tile([128, 8, F], fp32)
            wv_sb = const.tile([128, 8, F], fp32)
            wo_sb = const.tile([F, H], fp32)
            nc.sync.dma_start(wq_sb[:], wqT.rearrange("(t p) f -> p t f", p=128))
            nc.sync.dma_start(wk_sb[:], wkT.rearrange("(t p) f -> p t f", p=128))
            nc.sync.dma_start(wv_sb[:], wvT.rearrange("(t p) f -> p t f", p=128))
            nc.sync.dma_start(wo_sb[:], woT[:])
            bq_sb = const.tile([F, 1], fp32)
            bk_sb = const.tile([F, 1], fp32)
            bv_sb = const.tile([F, 1], fp32)
            nc.sync.dma_start(bq_sb[:], bqc[:])
            nc.sync.dma_start(bk_sb[:], bkc[:])
            nc.sync.dma_start(bv_sb[:], bvc[:])
            ones_sb = const.tile([1, 64], fp32)
            nc.vector.memset(ones_sb[:], 1.0)

            for b in range(B):
                mask_sb = maskp.tile([128, NKT], fp32)
                nc.sync.dma_start(
                    mask_sb[:], maskb[b, :].rearrange("(t p) -> p t", p=128)
                )

                qt = qtp.tile([128, S], fp32)
                kt = ktp.tile([128, S], fp32)
                # v: [128 part, 16 s-tiles, 130]; per s-tile cols 0:64 = V_h0,
                # 64 = 1.0, 65:129 = V_h1, 129 = 1.0 -> lhsT [V_h|1] slices.
                v = vp.tile([128, NKT, 130], fp32)
                nc.vector.memset(v[:, :, 64:65], 1.0)
                nc.vector.memset(v[:, :, 129:130], 1.0)
                ctxT = ctxp.tile([128, S], fp32)

                # --- projections (per s-block) ---
                for sblk in range(NSB):
                    cols = ds(b * S + sblk * SBLK, SBLK)
                    xt_t = xtp.tile([128, 8, SBLK], fp32)
                    nc.sync.dma_start(xt_t[:], xt_r[:, :, cols])

                    ps_q = ps_s.tile([128, SBLK], fp32, tag="s")
                    for ht in range(8):
                        nc.tensor.matmul(
                            ps_q[:], wq_sb[:, ht, :], xt_t[:, ht, :],
                            start=(ht == 0), stop=(ht == 7),
                        )
                    nc.scalar.activation(
                        qt[:, ts(sblk, SBLK)], ps_q[:], Act.Identity,
                        bias=bq_sb[:], scale=1.0,
                    )
                    ps_k = ps_s.tile([128, SBLK], fp32, tag="s")
                    for ht in range(8):
                        nc.tensor.matmul(
                            ps_k[:], wk_sb[:, ht, :], xt_t[:, ht, :],
                            start=(ht == 0), stop=(ht == 7),
                        )
                    nc.scalar.activation(
                        kt[:, ts(sblk, SBLK)], ps_k[:], Act.Identity,
                        bias=bk_sb[:], scale=1.0,
                    )
                    for st in range(4):
                        s_tile = sblk * 4 + st
                        ps_vt = ps_v.tile([128, 128], fp32)
                        for ht in range(8):
                            nc.tensor.matmul(
                                ps_vt[:], xt_t[:, ht, ts(st, 128)],
                                wv_sb[:, ht, :],
                                start=(ht == 0), stop=(ht == 7),
                            )
                        nc.scalar.copy(v[:, s_tile, 0:64], ps_vt[:, 0:64])
                        nc.scalar.copy(v[:, s_tile, 65:129], ps_vt[:, 64:128])

                # --- attention (2 heads row-tiled) ---
                for qblk in range(NSB):
                    qcols = ts(qblk, SBLK)
                    p_tiles = [[None] * NKT, [None] * NKT]
                    for kt_i in range(NKT):
                        for h in (0, 1):
                            hp = ds(h * 64, 64)
                            ps_t = ps_s.tile([128, SBLK], fp32, tag="s")
                            nc.tensor.matmul(
                                ps_t[:], kt[hp, ts(kt_i, 128)], qt[hp, qcols],
                                start=True, stop=True,
                            )
                            p_t = pp.tile([128, SBLK], fp32)
                            nc.scalar.activation(
                                p_t[:], ps_t[:], Act.Exp,
                                bias=mask_sb[:, kt_i : kt_i + 1],
                                scale=1.0 / np.sqrt(HD),
                            )
                            p_tiles[h][kt_i] = p_t
                    for h in (0, 1):
                        ps_c = ps_ctx.tile([65, SBLK], fp32)
                        for kt_i in range(NKT):
                            nc.tensor.matmul(
                                ps_c[:], v[:, kt_i, h * 65 : h * 65 + 65],
                                p_tiles[h][kt_i][:],
                                start=(kt_i == 0), stop=(kt_i == NKT - 1),
                            )
                        r = rp.tile([1, SBLK], fp32)
                        nc.vector.reciprocal(r[:], ps_c[64:65, :])
                        ps_bc = ps_b.tile([64, SBLK], fp32)
                        nc.tensor.matmul(
                            ps_bc[:], ones_sb[:], r[:], start=True, stop=True
                        )
                        cu = cup.tile([64, SBLK], fp32)
                        nc.scalar.copy(cu[:], ps_c[0:64, :])
                        dst = ctxT[ds(h * 64, 64), qcols]
                        nc.vector.tensor_tensor(dst, cu[:], ps_bc[:], Alu.mult)
                        if apply_bv:
                            nc.vector.tensor_scalar_add(
                                dst, dst, bv_sb[ds(h * 64, 64), :]
                            )

                # --- output projection partial ---
                for sblk in range(NSB):
                    for mt in range(8):
                        ps_ot = ps_o.tile([128, SBLK], fp32)
                        nc.tensor.matmul(
                            ps_ot[:], wo_sb[:, ts(mt, 128)],
                            ctxT[:, ts(sblk, SBLK)], start=True, stop=True,
                        )
                        ob = osb.tile([128, SBLK], fp32)
                        nc.vector.tensor_copy(ob[:], ps_ot[:])
                        nc.sync.dma_start(
                            outT[ts(mt, 128), ds(b * S + sblk * SBLK, SBLK)],
                            ob[:],
                        )

    nc.finalize()
    return nc


def _get_nc(apply_bv: bool):
    key = apply_bv
    if key not in _nc_cache:
        _nc_cache[key] = _build_nc(apply_bv)
    return _nc_cache[key]


def kernel(hidden_states, attention_mask, Wq, bq, Wk, bk, Wv, bv, Wo, bo):
    from concourse.bass_utils import run_bass_kernel_spmd

    hs = np.asarray(hidden_states, dtype=np.float32)
    mask = np.asarray(attention_mask, dtype=np.float32)
    Wq = np.asarray(Wq, dtype=np.float32)
    Wk = np.asarray(Wk, dtype=np.float32)
    Wv = np.asarray(Wv, dtype=np.float32)
    Wo = np.asarray(Wo, dtype=np.float32)
    bq = np.asarray(bq, dtype=np.float32)
    bk = np.asarray(bk, dtype=np.float32)
    bv = np.asarray(bv, dtype=np.float32)
    bo = np.asarray(bo, dtype=np.float32)

    XT = np.ascontiguousarray(hs.reshape(B * S, H).T)  # [H, B*S]
    maskb = np.ascontiguousarray(mask.reshape(B, S))

    apply_bv = bool(np.any(bv != 0.0))
    nc = _get_nc(apply_bv)

    in_maps = []
    for c in range(N_CORES):
        fc = slice(c * F, (c + 1) * F)
        in_maps.append(
            {
                "xt": XT,
                "wqT": np.ascontiguousarray(Wq[fc, :].T),
                "wkT": np.ascontiguousarray(Wk[fc, :].T),
                "wvT": np.ascontiguousarray(Wv[fc, :].T),
                "woT": np.ascontiguousarray(Wo[:, fc].T),
                "bqc": np.ascontiguousarray(bq[fc].reshape(F, 1)),
                "bkc": np.ascontiguousarray(bk[fc].reshape(F, 1)),
                "bvc": np.ascontiguousarray(bv[fc].reshape(F, 1)),
                "maskb": maskb,
            }
        )

    res = run_bass_kernel_spmd(nc, in_maps, list(range(N_CORES)))
    acc = res.results[0]["outT"].astype(np.float32)
    for c in range(1, N_CORES):
        acc = acc + res.results[c]["outT"]
    out = acc.T + bo  # [B*S, H]
    return np.ascontiguousarray(out.reshape(B, S, H), dtype=np.float32)


# revision 9
# speedup vs baseline: 2.7500x; 2.7500x over previous
"""Multi-head attention (B=4, S=2048, H=1024, NH=16) on 8 trn2 NeuronCores.

Sharding: tensor-parallel over heads. Core c owns heads 2c, 2c+1 (feature
columns 128c:128c+128 of Q/K/V). Each core computes its head-slice
projections from the full (host-pre-transposed) X^T, attention for its 8
(batch, head) pairs, and a rank-128 partial of the output projection.
Host sums the 8 partial O^T arrays, transposes back, and adds bo.

Device-side layout notes:
 - All matmul operands use dt.float32r (full PE rate vs 4 cycles/row for
   fp32; measured l2 err ~1.5e-4 per K=1024 contraction). Non-matmul
   engines access those tiles through .bitcast(float32).
 - Activations stay "transposed" (feature on partitions): Q^T/K^T are
   [128, 2048] per batch; scores are computed as S^T = K Q^T with
   k-positions on partitions so the additive mask is a per-partition bias
   fused into the ACT exp (softmax denominators come from an extra ones
   column appended to V: the PV matmul's 65th output row).
 - The two heads' QK matmuls (contraction = head_dim = 64) are row-tiled
   into PE partition halves 0:64 / 64:128 so they run concurrently.
 - V is produced as V^T (N=512 matmuls) then PE-transposed per 128-tile.
 - Normalization: reciprocal of denom row -> broadcast over 64 partitions
   via a K=1 matmul with a ones stationary -> DVE multiply.
"""

import numpy as np

H = 1024
NH = 16
HD = 64
B = 4
S = 2048
N_CORES = 8
F = H // N_CORES  # 128 features (2 heads) per core
SBLK = 512  # s-block (moving-operand free dim, fp32 max)
NSB = S // SBLK  # 4 s-blocks per batch
NKT = S // 128  # 16 k-position tiles per batch

_nc_cache = {}


def _build_nc(apply_bv: bool):
    import concourse.bacc as bacc
    import concourse.tile as tile
    from concourse import mybir
    from concourse.bass import ts, ds
    from concourse.masks import make_identity
    from contextlib import ExitStack

    fp32 = mybir.dt.float32
    f32r = mybir.dt.float32r
    Act = mybir.ActivationFunctionType
    Alu = mybir.AluOpType

    def c(ap):  # fp32 view of an f32r tile for non-matmul engines
        return ap.bitcast(fp32)

    nc = bacc.Bacc("TRN2", target_bir_lowering=False)

    xt = nc.dram_tensor("xt", [H, B * S], f32r, kind="ExternalInput")
    wqT = nc.dram_tensor("wqT", [H, F], f32r, kind="ExternalInput")
    wkT = nc.dram_tensor("wkT", [H, F], f32r, kind="ExternalInput")
    wvT = nc.dram_tensor("wvT", [H, F], f32r, kind="ExternalInput")
    woT = nc.dram_tensor("woT", [F, H], f32r, kind="ExternalInput")
    bqc = nc.dram_tensor("bqc", [F, 1], fp32, kind="ExternalInput")
    bkc = nc.dram_tensor("bkc", [F, 1], fp32, kind="ExternalInput")
    bvc = nc.dram_tensor("bvc", [F, 1], fp32, kind="ExternalInput")
    maskb = nc.dram_tensor("maskb", [B, S], fp32, kind="ExternalInput")
    outT = nc.dram_tensor("outT", [H, B * S], fp32, kind="ExternalOutput")

    xt_r = xt.rearrange("(t p) n -> p t n", p=128)  # [128, 8, 8192]

    with tile.TileContext(nc) as tc, ExitStack() as es:
        const = es.enter_context(tc.tile_pool(name="const", bufs=1))
        maskp = es.enter_context(tc.tile_pool(name="maskp", bufs=2))
        xtp = es.enter_context(tc.tile_pool(name="xtp", bufs=2))
        qtp = es.enter_context(tc.tile_pool(name="qtp", bufs=2))
        ktp = es.enter_context(tc.tile_pool(name="ktp", bufs=2))
        vtp = es.enter_context(tc.tile_pool(name="vtp", bufs=3))
        vp = es.enter_context(tc.tile_pool(name="vp", bufs=2))
        pp = es.enter_context(tc.tile_pool(name="pp", bufs=18))
        ctxp = es.enter_context(tc.tile_pool(name="ctxp", bufs=2))
        cup = es.enter_context(tc.tile_pool(name="cup", bufs=4))
        rp = es.enter_context(tc.tile_pool(name="rp", bufs=4))
        osb = es.enter_context(tc.tile_pool(name="osb", bufs=4))
        ps_s = es.enter_context(tc.tile_pool(name="ps_s", bufs=3, space="PSUM"))
        ps_cb = es.enter_context(tc.tile_pool(name="ps_cb", bufs=3, space="PSUM"))
        ps_ovt = es.enter_context(tc.tile_pool(name="ps_ovt", bufs=2, space="PSUM"))

        # --- constants ---
        wq_sb = const.tile([128, 8, F], f32r)
        wk_sb = const.tile([128, 8, F], f32r)
        wv_sb = const.tile([128, 8, F], f32r)
        wo_sb = const.tile([F, H], f32r)
        nc.sync.dma_start(wq_sb[:], wqT.rearrange("(t p) f -> p t f", p=128))
        nc.sync.dma_start(wk_sb[:], wkT.rearrange("(t p) f -> p t f", p=128))
        nc.sync.dma_start(wv_sb[:], wvT.rearrange("(t p) f -> p t f", p=128))
        nc.sync.dma_start(wo_sb[:], woT[:])
        bq_sb = const.tile([F, 1], fp32)
        bk_sb = const.tile([F, 1], fp32)
        bv_sb = const.tile([F, 1], fp32)
        nc.sync.dma_start(bq_sb[:], bqc[:])
        nc.sync.dma_start(bk_sb[:], bkc[:])
        nc.sync.dma_start(bv_sb[:], bvc[:])
        ones32 = const.tile([1, 64], fp32)
        nc.vector.memset(ones32[:], 1.0)
        ones_sb = const.tile([1, 64], f32r)
        nc.vector.tensor_copy(ones_sb[:], ones32[:])
        vones32 = const.tile([128, NKT, 1], fp32)
        nc.vector.memset(vones32[:], 1.0)
        ident32 = const.tile([128, 128], fp32)
        make_identity(nc, ident32[:])
        ident = const.tile([128, 128], f32r)
        nc.vector.tensor_copy(ident[:], ident32[:])

        for b in range(B):
            mask_sb = maskp.tile([128, NKT], fp32)
            nc.sync.dma_start(
                mask_sb[:], maskb[b, :].rearrange("(t p) -> p t", p=128)
            )

            qt = qtp.tile([128, S], f32r)
            kt = ktp.tile([128, S], f32r)
            # v: [128 part, 16 s-tiles, 130]; per s-tile cols 0:64 = V_h0,
            # 64 = 1.0, 65:129 = V_h1, 129 = 1.0 -> lhsT [V_h|1] slices.
            v = vp.tile([128, NKT, 130], f32r)
            nc.vector.tensor_copy(v[:, :, 64:65], vones32[:])
            nc.vector.tensor_copy(v[:, :, 129:130], vones32[:])
            ctxT = ctxp.tile([128, S], f32r)

            # --- projections (per s-block) ---
            for sblk in range(NSB):
                cols = ds(b * S + sblk * SBLK, SBLK)
                xt_t = xtp.tile([128, 8, SBLK], f32r)
                nc.sync.dma_start(xt_t[:], xt_r[:, :, cols])

                ps_q = ps_s.tile([128, SBLK], fp32, tag="s")
                for ht in range(8):
                    nc.tensor.matmul(
                        ps_q[:], wq_sb[:, ht, :], xt_t[:, ht, :],
                        start=(ht == 0), stop=(ht == 7),
                    )
                nc.vector.tensor_scalar_add(
                    qt[:, ts(sblk, SBLK)], ps_q[:], bq_sb[:]
                )
                ps_k = ps_s.tile([128, SBLK], fp32, tag="s")
                for ht in range(8):
                    nc.tensor.matmul(
                        ps_k[:], wk_sb[:, ht, :], xt_t[:, ht, :],
                        start=(ht == 0), stop=(ht == 7),
                    )
                nc.vector.tensor_scalar_add(
                    kt[:, ts(sblk, SBLK)], ps_k[:], bk_sb[:]
                )
                # V^T then per-128-tile PE transpose into v
                ps_vT = ps_s.tile([128, SBLK], fp32, tag="s")
                for ht in range(8):
                    nc.tensor.matmul(
                        ps_vT[:], wv_sb[:, ht, :], xt_t[:, ht, :],
                        start=(ht == 0), stop=(ht == 7),
                    )
                vT_sb = vtp.tile([128, SBLK], f32r)
                nc.vector.tensor_copy(vT_sb[:], ps_vT[:])
                for st in range(4):
                    s_tile = sblk * 4 + st
                    ps_tr = ps_ovt.tile([128, 128], f32r, tag="ovt")
                    nc.tensor.transpose(
                        ps_tr[:], vT_sb[:, ts(st, 128)], ident[:]
                    )
                    nc.vector.tensor_copy(
                        v[:, s_tile, 0:64], ps_tr[:, 0:64]
                    )
                    nc.vector.tensor_copy(
                        v[:, s_tile, 65:129], ps_tr[:, 64:128]
                    )

            # --- attention (2 heads row-tiled) ---
            for qblk in range(NSB):
                qcols = ts(qblk, SBLK)
                p_tiles = [[None] * NKT, [None] * NKT]
                for kt_i in range(NKT):
                    for h in (0, 1):
                        hp = ds(h * 64, 64)
                        ps_t = ps_s.tile([128, SBLK], fp32, tag="s")
                        nc.tensor.matmul(
                            ps_t[:], kt[hp, ts(kt_i, 128)], qt[hp, qcols],
                            start=True, stop=True,
                        )
                        p_t = pp.tile([128, SBLK], f32r)
                        nc.scalar.activation(
                            p_t[:], ps_t[:], Act.Exp,
                            bias=mask_sb[:, kt_i : kt_i + 1],
                            scale=1.0 / np.sqrt(HD),
                        )
                        p_tiles[h][kt_i] = p_t
                for h in (0, 1):
                    ps_c = ps_cb.tile([65, SBLK], fp32, tag="cb")
                    for kt_i in range(NKT):
                        nc.tensor.matmul(
                            ps_c[:], v[:, kt_i, h * 65 : h * 65 + 65],
                            p_tiles[h][kt_i][:],
                            start=(kt_i == 0), stop=(kt_i == NKT - 1),
                        )
                    r = rp.tile([1, SBLK], f32r)
                    with nc.allow_low_precision(
                        reason="fp32r softmax denominators"
                    ):
                        nc.vector.reciprocal(r[:], ps_c[64:65, :])
                    ps_bc = ps_cb.tile([64, SBLK], fp32, tag="cb")
                    nc.tensor.matmul(
                        ps_bc[:], ones_sb[:], r[:], start=True, stop=True
                    )
                    cu = cup.tile([64, SBLK], fp32)
                    nc.vector.tensor_copy(cu[:], ps_c[0:64, :])
                    dst = ctxT[ds(h * 64, 64), qcols]
                    nc.vector.tensor_tensor(dst, cu[:], ps_bc[:], Alu.mult)
                    if apply_bv:
                        nc.vector.tensor_scalar_add(
                            dst, dst, bv_sb[ds(h * 64, 64), :]
                        )

            # --- output projection partial ---
            for sblk in range(NSB):
                for mt in range(8):
                    ps_ot = ps_ovt.tile([128, SBLK], fp32, tag="ovt")
                    nc.tensor.matmul(
                        ps_ot[:], wo_sb[:, ts(mt, 128)],
                        ctxT[:, ts(sblk, SBLK)], start=True, stop=True,
                    )
                    ob = osb.tile([128, SBLK], fp32)
                    nc.vector.tensor_copy(ob[:], ps_ot[:])
                    nc.sync.dma_start(
                        outT[ts(mt, 128), ds(b * S + sblk * SBLK, SBLK)],
                        ob[:],
                    )

    nc.finalize()
    return nc


def _get_nc(apply_bv: bool):
    key = apply_bv
    if key not in _nc_cache:
        _nc_cache[key] = _build_nc(apply_bv)
    return _nc_cache[key]


def kernel(hidden_states, attention_mask, Wq, bq, Wk, bk, Wv, bv, Wo, bo):
    from concourse.bass_utils import run_bass_kernel_spmd

    hs = np.asarray(hidden_states, dtype=np.float32)
    mask = np.asarray(attention_mask, dtype=np.float32)
    Wq = np.asarray(Wq, dtype=np.float32)
    Wk = np.asarray(Wk, dtype=np.float32)
    Wv = np.asarray(Wv, dtype=np.float32)
    Wo = np.asarray(Wo, dtype=np.float32)
    bq = np.asarray(bq, dtype=np.float32)
    bk = np.asarray(bk, dtype=np.float32)
    bv = np.asarray(bv, dtype=np.float32)
    bo = np.asarray(bo, dtype=np.float32)

    XT = np.ascontiguousarray(hs.reshape(B * S, H).T)  # [H, B*S]
    maskb = np.ascontiguousarray(mask.reshape(B, S))

    apply_bv = bool(np.any(bv != 0.0))
    nc = _get_nc(apply_bv)

    in_maps = []
    for c in range(N_CORES):
        fc = slice(c * F, (c + 1) * F)
        in_maps.append(
            {
                "xt": XT,
                "wqT": np.ascontiguousarray(Wq[fc, :].T),
                "wkT": np.ascontiguousarray(Wk[fc, :].T),
                "wvT": np.ascontiguousarray(Wv[fc, :].T),
                "woT": np.ascontiguousarray(Wo[:, fc].T),
                "bqc": np.ascontiguousarray(bq[fc].reshape(F, 1)),
                "bkc": np.ascontiguousarray(bk[fc].reshape(F, 1)),
                "bvc": np.ascontiguousarray(bv[fc].reshape(F, 1)),
                "maskb": maskb,
            }
        )

    res = run_bass_kernel_spmd(nc, in_maps, list(range(N_CORES)))
    acc = res.results[0]["outT"].astype(np.float32)
    for c in range(1, N_CORES):
        acc = acc + res.results[c]["outT"]
    out = acc.T + bo  # [B*S, H]
    return np.ascontiguousarray(out.reshape(B, S, H), dtype=np.float32)


# revision 13
# speedup vs baseline: 3.2425x; 1.1791x over previous
"""Multi-head attention (B=4, S=2048, H=1024, NH=16) on 8 trn2 NeuronCores.

Sharding: tensor-parallel over heads. Core c owns heads 2c, 2c+1 (feature
columns 128c:128c+128 of Q/K/V). Each core computes its head-slice
projections from the full (host-pre-transposed) X^T, attention for its 8
(batch, head) pairs, and a rank-128 partial of the output projection.
Host sums the 8 partial O^T arrays, transposes back, and adds bo.

Device-side layout notes:
 - All matmul operands use dt.float32r (full PE rate vs 4 cycles/row for
   fp32; measured l2 err ~1.5e-4 per K=1024 contraction). Non-matmul
   engines access those tiles through .bitcast(float32).
 - Activations stay "transposed" (feature on partitions): Q^T/K^T are
   [128, 2048] per batch; scores are computed as S^T = K Q^T with
   k-positions on partitions so the additive mask is a per-partition bias
   fused into the ACT exp (softmax denominators come from an extra ones
   column appended to V: the PV matmul's 65th output row).
 - The two heads' QK matmuls (contraction = head_dim = 64) are row-tiled
   into PE partition halves 0:64 / 64:128 so they run concurrently.
 - V is produced as V^T (N=512 matmuls) then PE-transposed per 128-tile.
 - Normalization: reciprocal of denom row -> broadcast over 64 partitions
   via a K=1 matmul with a ones stationary -> DVE multiply.
"""

import numpy as np

H = 1024
NH = 16
HD = 64
B = 4
S = 2048
N_CORES = 8
F = H // N_CORES  # 128 features (2 heads) per core
SBLK = 512  # s-block (moving-operand free dim, fp32 max)
NSB = S // SBLK  # 4 s-blocks per batch
NKT = S // 128  # 16 k-position tiles per batch

_nc_cache = {}


def _build_nc(apply_bv: bool):
    import concourse.bacc as bacc
    import concourse.tile as tile
    from concourse import mybir
    from concourse.bass import ts, ds
    from concourse.masks import make_identity
    from contextlib import ExitStack

    fp32 = mybir.dt.float32
    f32r = mybir.dt.float32r
    Act = mybir.ActivationFunctionType
    Alu = mybir.AluOpType

    def c(ap):  # fp32 view of an f32r tile for non-matmul engines
        return ap.bitcast(fp32)

    nc = bacc.Bacc("TRN2", target_bir_lowering=False)

    xt = nc.dram_tensor("xt", [H, B * S], f32r, kind="ExternalInput")
    wqT = nc.dram_tensor("wqT", [H, F], f32r, kind="ExternalInput")
    wkT = nc.dram_tensor("wkT", [H, F], f32r, kind="ExternalInput")
    wvT = nc.dram_tensor("wvT", [H, F], f32r, kind="ExternalInput")
    woT = nc.dram_tensor("woT", [F, H], f32r, kind="ExternalInput")
    bqc = nc.dram_tensor("bqc", [F, 1], fp32, kind="ExternalInput")
    bkc = nc.dram_tensor("bkc", [F, 1], fp32, kind="ExternalInput")
    bvc = nc.dram_tensor("bvc", [F, 1], fp32, kind="ExternalInput")
    maskb = nc.dram_tensor("maskb", [B, S], fp32, kind="ExternalInput")
    outT = nc.dram_tensor("outT", [H, B * S], fp32, kind="ExternalOutput")

    xt_r = xt.rearrange("(t p) n -> p t n", p=128)  # [128, 8, 8192]

    with tile.TileContext(nc) as tc, ExitStack() as es:
        const = es.enter_context(tc.tile_pool(name="const", bufs=1))
        maskp = es.enter_context(tc.tile_pool(name="maskp", bufs=2))
        xtp = es.enter_context(tc.tile_pool(name="xtp", bufs=3))
        qtp = es.enter_context(tc.tile_pool(name="qtp", bufs=2))
        ktp = es.enter_context(tc.tile_pool(name="ktp", bufs=2))
        vtp = es.enter_context(tc.tile_pool(name="vtp", bufs=3))
        vp = es.enter_context(tc.tile_pool(name="vp", bufs=2))
        pp = es.enter_context(tc.tile_pool(name="pp", bufs=6))
        ctxp = es.enter_context(tc.tile_pool(name="ctxp", bufs=2))
        cup = es.enter_context(tc.tile_pool(name="cup", bufs=4))
        rp = es.enter_context(tc.tile_pool(name="rp", bufs=4))
        osb = es.enter_context(tc.tile_pool(name="osb", bufs=4))
        ps_s = es.enter_context(tc.tile_pool(name="ps_s", bufs=2, space="PSUM"))
        ps_cb = es.enter_context(tc.tile_pool(name="ps_cb", bufs=2, space="PSUM"))
        ps_ovt = es.enter_context(tc.tile_pool(name="ps_ovt", bufs=2, space="PSUM"))

        # --- constants ---
        wq_sb = const.tile([128, 8, F], f32r)
        wk_sb = const.tile([128, 8, F], f32r)
        wv_sb = const.tile([128, 8, F], f32r)
        wo_sb = const.tile([F, H], f32r)
        nc.sync.dma_start(wq_sb[:], wqT.rearrange("(t p) f -> p t f", p=128))
        nc.sync.dma_start(wk_sb[:], wkT.rearrange("(t p) f -> p t f", p=128))
        nc.sync.dma_start(wv_sb[:], wvT.rearrange("(t p) f -> p t f", p=128))
        nc.sync.dma_start(wo_sb[:], woT[:])
        bq_sb = const.tile([F, 1], fp32)
        bk_sb = const.tile([F, 1], fp32)
        bv_sb = const.tile([F, 1], fp32)
        nc.sync.dma_start(bq_sb[:], bqc[:])
        nc.sync.dma_start(bk_sb[:], bkc[:])
        nc.sync.dma_start(bv_sb[:], bvc[:])
        ones32 = const.tile([1, 64], fp32)
        nc.vector.memset(ones32[:], 1.0)
        ones_sb = const.tile([1, 64], f32r)
        nc.vector.tensor_copy(ones_sb[:], ones32[:])
        vones32 = const.tile([128, NKT, 1], fp32)
        nc.vector.memset(vones32[:], 1.0)
        ident32 = const.tile([128, 128], fp32)
        make_identity(nc, ident32[:])
        ident = const.tile([128, 128], f32r)
        nc.vector.tensor_copy(ident[:], ident32[:])

        for b in range(B):
            mask_sb = maskp.tile([128, NKT], fp32)
            nc.sync.dma_start(
                mask_sb[:], maskb[b, :].rearrange("(t p) -> p t", p=128)
            )

            qt = qtp.tile([128, S], f32r)
            kt = ktp.tile([128, S], f32r)
            # v: [128 part, 16 s-tiles, 130]; per s-tile cols 0:64 = V_h0,
            # 64 = 1.0, 65:129 = V_h1, 129 = 1.0 -> lhsT [V_h|1] slices.
            v = vp.tile([128, NKT, 130], f32r)
            nc.vector.tensor_copy(v[:, :, 64:65], vones32[:])
            nc.vector.tensor_copy(v[:, :, 129:130], vones32[:])
            ctxT = ctxp.tile([128, S], f32r)

            # --- projections (per s-block) ---
            for sblk in range(NSB):
                cols = ds(b * S + sblk * SBLK, SBLK)
                xt_t = xtp.tile([128, 8, SBLK], f32r)
                nc.sync.dma_start(xt_t[:, 0:4, :], xt_r[:, 0:4, cols])
                nc.sync.dma_start(xt_t[:, 4:8, :], xt_r[:, 4:8, cols])

                ps_qk = ps_s.tile([128, 2 * SBLK], fp32, tag="s")
                for ht in range(8):
                    nc.tensor.matmul(
                        ps_qk[:, 0:SBLK], wq_sb[:, ht, :], xt_t[:, ht, :],
                        start=(ht == 0), stop=(ht == 7),
                    )
                for ht in range(8):
                    nc.tensor.matmul(
                        ps_qk[:, SBLK : 2 * SBLK], wk_sb[:, ht, :],
                        xt_t[:, ht, :],
                        start=(ht == 0), stop=(ht == 7),
                    )
                nc.vector.tensor_scalar_add(
                    qt[:, ts(sblk, SBLK)], ps_qk[:, 0:SBLK], bq_sb[:]
                )
                nc.vector.tensor_scalar_add(
                    kt[:, ts(sblk, SBLK)], ps_qk[:, SBLK : 2 * SBLK], bk_sb[:]
                )
                # V^T then per-128-tile PE transpose into v
                ps_vT = ps_s.tile([128, 2 * SBLK], fp32, tag="s")
                for ht in range(8):
                    nc.tensor.matmul(
                        ps_vT[:, 0:SBLK], wv_sb[:, ht, :], xt_t[:, ht, :],
                        start=(ht == 0), stop=(ht == 7),
                    )
                vT_sb = vtp.tile([128, SBLK], f32r)
                nc.vector.tensor_copy(vT_sb[:], ps_vT[:, 0:SBLK])
                for st in range(4):
                    s_tile = sblk * 4 + st
                    ps_tr = ps_ovt.tile([128, 128], f32r, tag="ovt")
                    nc.tensor.transpose(
                        ps_tr[:], vT_sb[:, ts(st, 128)], ident[:]
                    )
                    nc.vector.tensor_copy(
                        v[:, s_tile, 0:130].rearrange(
                            "p (two x) -> p two x", x=65
                        )[:, :, 0:64],
                        ps_tr[:].rearrange("p (two x) -> p two x", x=64),
                    )

            # --- attention (2 heads row-tiled, PV interleaved with QK) ---
            for qblk in range(NSB):
                qcols = ts(qblk, SBLK)
                ps_c2 = [
                    ps_cb.tile([65, SBLK], fp32, tag="cb", name=f"ps_c{h}")
                    for h in (0, 1)
                ]
                for kt_i in range(NKT):
                    ps_t = ps_s.tile([128, 2 * SBLK], fp32, tag="s")
                    for h in (0, 1):
                        hp = ds(h * 64, 64)
                        nc.tensor.matmul(
                            ps_t[:, h * SBLK : (h + 1) * SBLK],
                            kt[hp, ts(kt_i, 128)], qt[hp, qcols],
                            start=True, stop=True,
                        )
                    p_t = pp.tile([128, 2 * SBLK], f32r)
                    nc.scalar.activation(
                        p_t[:], ps_t[:], Act.Exp,
                        bias=mask_sb[:, kt_i : kt_i + 1],
                        scale=1.0 / np.sqrt(HD),
                    )
                    for h in (0, 1):
                        nc.tensor.matmul(
                            ps_c2[h][:], v[:, kt_i, h * 65 : h * 65 + 65],
                            p_t[:, h * SBLK : (h + 1) * SBLK],
                            start=(kt_i == 0), stop=(kt_i == NKT - 1),
                        )
                for h in (0, 1):
                    ps_c = ps_c2[h]
                    r = rp.tile([1, SBLK], f32r)
                    with nc.allow_low_precision(
                        reason="fp32r softmax denominators"
                    ):
                        nc.vector.reciprocal(r[:], ps_c[64:65, :])
                    ps_bc = ps_ovt.tile([64, SBLK], fp32, tag="ovt")
                    nc.tensor.matmul(
                        ps_bc[:], ones_sb[:], r[:], start=True, stop=True
                    )
                    cu = cup.tile([64, SBLK], fp32)
                    nc.vector.tensor_copy(cu[:], ps_c[0:64, :])
                    dst = ctxT[ds(h * 64, 64), qcols]
                    nc.vector.tensor_tensor(dst, cu[:], ps_bc[:], Alu.mult)
                    if apply_bv:
                        nc.vector.tensor_scalar_add(
                            dst, dst, bv_sb[ds(h * 64, 64), :]
                        )

                # --- output projection partial for this q-block ---
                for mt in range(8):
                    ps_ot = ps_ovt.tile([128, SBLK], fp32, tag="ovt")
                    nc.tensor.matmul(
                        ps_ot[:], wo_sb[:, ts(mt, 128)],
                        ctxT[:, qcols], start=True, stop=True,
                    )
                    ob = osb.tile([128, SBLK], fp32)
                    nc.vector.tensor_copy(ob[:], ps_ot[:])
                    nc.sync.dma_start(
                        outT[ts(mt, 128), ds(b * S + qblk * SBLK, SBLK)],
                        ob[:],
                    )

    nc.finalize()
    return nc


def _get_nc(apply_bv: bool):
    key = apply_bv
    if key not in _nc_cache:
        _nc_cache[key] = _build_nc(apply_bv)
    return _nc_cache[key]


def kernel(hidden_states, attention_mask, Wq, bq, Wk, bk, Wv, bv, Wo, bo):
    from concourse.bass_utils import run_bass_kernel_spmd

    hs = np.asarray(hidden_states, dtype=np.float32)
    mask = np.asarray(attention_mask, dtype=np.float32)
    Wq = np.asarray(Wq, dtype=np.float32)
    Wk = np.asarray(Wk, dtype=np.float32)
    Wv = np.asarray(Wv, dtype=np.float32)
    Wo = np.asarray(Wo, dtype=np.float32)
    bq = np.asarray(bq, dtype=np.float32)
    bk = np.asarray(bk, dtype=np.float32)
    bv = np.asarray(bv, dtype=np.float32)
    bo = np.asarray(bo, dtype=np.float32)

    XT = np.ascontiguousarray(hs.reshape(B * S, H).T)  # [H, B*S]
    maskb = np.ascontiguousarray(mask.reshape(B, S))

    apply_bv = bool(np.any(bv != 0.0))
    nc = _get_nc(apply_bv)

    in_maps = []
    for c in range(N_CORES):
        fc = slice(c * F, (c + 1) * F)
        in_maps.append(
            {
                "xt": XT,
                "wqT": np.ascontiguousarray(Wq[fc, :].T),
                "wkT": np.ascontiguousarray(Wk[fc, :].T),
                "wvT": np.ascontiguousarray(Wv[fc, :].T),
                "woT": np.ascontiguousarray(Wo[:, fc].T),
                "bqc": np.ascontiguousarray(bq[fc].reshape(F, 1)),
                "bkc": np.ascontiguousarray(bk[fc].reshape(F, 1)),
                "bvc": np.ascontiguousarray(bv[fc].reshape(F, 1)),
                "maskb": maskb,
            }
        )

    res = run_bass_kernel_spmd(nc, in_maps, list(range(N_CORES)))
    acc = res.results[0]["outT"].astype(np.float32)
    for c in range(1, N_CORES):
        acc = acc + res.results[c]["outT"]
    out = acc.T + bo  # [B*S, H]
    return np.ascontiguousarray(out.reshape(B, S, H), dtype=np.float32)
